# revision 11
# baseline (speedup 1.0000x reference)
"""AdaptivePConv Trainium2 kernel (8 NeuronCores, data-parallel over batch).

Per core (sample b = core index), on-device:
  1. per-channel min/max -> exact bins q = clip(trunc((x-mn)*recip(mx-mn+eps)*256), 0, 255)
     (recip is bit-identical to IEEE 1/d on this HW; trunc via magic-number
      round-to-nearest + compare fixup -> exact floor)
  2. per-channel 256-bin histogram via 255 fused threshold-count passes
     split across DVE (tensor_scalar is_ge + add-accum) and ACT (Sign + accum)
  3. entropy per channel (ACT Ln), top-16 channels via max8/match_replace
  4. selected channels -> 3x3 conv as 9-tap K=16 matmuls on PE
  5. unselected channels copied to output tail in ascending order via
     indirect-scatter DMA (selected rows skipped via bounds check)
"""
import os
import sys
import numpy as np

sys.path.insert(0, "/opt/trn_rl_repo")

import concourse.bass as bass
import concourse.bacc as bacc
import concourse.tile as tile
from concourse import mybir
import concourse.bass_utils as bu

F32 = mybir.dt.float32
BF16 = mybir.dt.bfloat16
I32 = mybir.dt.int32
U32 = mybir.dt.uint32
Alu = mybir.AluOpType
Act = mybir.ActivationFunctionType
AxX = mybir.AxisListType.X

B, C, H, W = 8, 64, 256, 256
N = H * W                  # 65536 pixels per channel
NHALF = N // 2             # 32768 per (channel, half) partition
FQ = 4096                  # free-size per histogram instruction
NQI = NHALF // FQ          # 4 instructions per threshold per engine
OC, P_SEL = 64, 16
C_OUT = OC + C - P_SEL     # 112
NBINS = 256
KD = 115                   # thresholds 0..KD-1 on DVE; KD..254 on ACT
MAGIC = float(np.float32(2.0 ** 23))
XCH = 1024                 # prepass chunk


def bcast(ap_small, ap_big):
    return bass.broadcast_tensor_aps(ap_small, ap_big)[0]


def build():
    nc = bacc.Bacc()
    x_ext = nc.declare_dram_parameter("x", [C, N], F32, isOutput=False)
    w_ext = nc.declare_dram_parameter("w", [16, 9 * OC], F32, isOutput=False)
    bias_ext = nc.declare_dram_parameter("bias", [OC, 1], F32, isOutput=False)
    biasA_ext = nc.declare_dram_parameter("biasA", [128, 255], F32, isOutput=False)
    blk_ext = nc.declare_dram_parameter("blkvec", [128, 1], F32, isOutput=False)
    iota_ext = nc.declare_dram_parameter("colio", [1, 64], F32, isOutput=False)
    out_ext = nc.declare_dram_parameter("out", [C_OUT, N], F32, isOutput=True)
    dbg_act = nc.declare_dram_parameter("dbg_act", [C, 1], F32, isOutput=True)
    dbg_idx = nc.declare_dram_parameter("dbg_idx", [1, 16], U32, isOutput=True)
    dbg_n = nc.declare_dram_parameter("dbg_n", [C, NBINS], F32, isOutput=True)

    scr_stat = nc.dram_tensor("scr_stat", [128], F32)
    scr_stat2 = nc.dram_tensor("scr_stat2", [64], F32)
    scr_cge = nc.dram_tensor("scr_cge", [128, 255], F32)
    scr_act = nc.dram_tensor("scr_act", [64], F32)
    scr_idx = nc.dram_tensor("scr_idx", [16], F32)
    scr_slot = nc.dram_tensor("scr_slot", [64], F32)

    def dram_ap(t, offset, pattern):
        return bass.AP(t, offset, pattern)

    with tile.TileContext(nc) as tc:
        with tc.tile_pool(name="persist", bufs=1) as pp:
            # ---- persistent small tiles ----
            wsbf = pp.tile([16, 9 * OC], F32)
            nc.sync.dma_start(wsbf[:], w_ext[:])
            wsb = pp.tile([16, 9 * OC], BF16)
            nc.vector.tensor_copy(wsb[:], wsbf[:])
            biasC = pp.tile([OC, 1], F32)
            nc.sync.dma_start(biasC[:], bias_ext[:])
            biasA = pp.tile([128, 255], F32)
            nc.sync.dma_start(biasA[:], biasA_ext[:])
            colio = pp.tile([1, 64], F32)
            nc.sync.dma_start(colio[:], iota_ext[:])
            epsT = pp.tile([128, 1], F32)
            nc.vector.memset(epsT[:], 1e-8)
            zb = pp.tile([128, 1], F32)
            nc.vector.memset(zb[:], 0.0)
            z64 = pp.tile([1, 64], F32)
            nc.vector.memset(z64[:], 0.0)

            mn128 = pp.tile([128, 1], F32)
            r256 = pp.tile([128, 1], F32)
            accD = pp.tile([128, NQI * KD], F32)
            accA = pp.tile([128, NQI * (255 - KD)], F32)

            with tc.tile_pool(name="qpool", bufs=1) as qp:
                q_sb = qp.tile([128, NHALF], F32)

                # ============ phase 1: min/max ============
                nch = NHALF // XCH
                with tc.tile_pool(name="xminmax", bufs=2) as xp:
                    mxP = pp.tile([128, nch], F32)
                    mnP = pp.tile([128, nch], F32)
                    for h in range(nch):
                        xh = xp.tile([128, XCH], F32, tag="xh")
                        nc.sync.dma_start(
                            xh[:],
                            x_ext[:].rearrange("c (t m) -> (c t) m", t=2)[:, h * XCH:(h + 1) * XCH])
                        nc.vector.tensor_reduce(mxP[:, h:h + 1], xh[:], axis=AxX, op=Alu.max)
                        nc.vector.tensor_reduce(mnP[:, h:h + 1], xh[:], axis=AxX, op=Alu.min)
                    mx1 = pp.tile([128, 1], F32)
                    mn1 = pp.tile([128, 1], F32)
                    nc.vector.tensor_reduce(mx1[:], mxP[:], axis=AxX, op=Alu.max)
                    nc.vector.tensor_reduce(mn1[:], mnP[:], axis=AxX, op=Alu.min)

                    # pair-combine (c,half) partitions via DRAM bounce, expand back
                    mx128 = pp.tile([128, 1], F32)
                    for src1, dst, op in ((mx1, mx128, Alu.max), (mn1, mn128, Alu.min)):
                        nc.sync.dma_start(dram_ap(scr_stat, 0, [[1, 128]]), src1[:])
                        ev = pp.tile([64, 1], F32, tag="ev")
                        od = pp.tile([64, 1], F32, tag="od")
                        nc.sync.dma_start(ev[:], dram_ap(scr_stat, 0, [[2, 64], [1, 1]]))
                        nc.sync.dma_start(od[:], dram_ap(scr_stat, 1, [[2, 64], [1, 1]]))
                        cmb = pp.tile([64, 1], F32, tag="cmb")
                        nc.vector.tensor_tensor(cmb[:], ev[:], od[:], op=op)
                        nc.sync.dma_start(dram_ap(scr_stat2, 0, [[1, 64]]), cmb[:])
                        nc.sync.dma_start(dst[:], dram_ap(scr_stat2, 0, [[1, 64], [0, 2]]))

                    dT = pp.tile([128, 1], F32)
                    nc.vector.scalar_tensor_tensor(dT[:], mx128[:], mn128[:], epsT[:],
                                                   op0=Alu.subtract, op1=Alu.add)
                    rT = pp.tile([128, 1], F32)
                    nc.vector.reciprocal(rT[:], dT[:])
                    nc.vector.tensor_scalar(r256[:], rT[:], 256.0, None, op0=Alu.mult)

                # ============ phase 2: q (exact trunc bins, fp32) ============
                with tc.tile_pool(name="xq", bufs=1) as xp2:
                    for h in range(nch):
                        sl = slice(h * XCH, (h + 1) * XCH)
                        xh = xp2.tile([128, XCH], F32, tag="xh2")
                        nc.sync.dma_start(
                            xh[:],
                            x_ext[:].rearrange("c (t m) -> (c t) m", t=2)[:, sl])
                        t_ = xp2.tile([128, XCH], F32, tag="t_")
                        nc.vector.scalar_tensor_tensor(t_[:], xh[:], mn128[:], bcast(r256[:], xh[:]),
                                                       op0=Alu.subtract, op1=Alu.mult)
                        y_ = xp2.tile([128, XCH], F32, tag="y_")
                        nc.vector.tensor_scalar(y_[:], t_[:], MAGIC, None, op0=Alu.add)
                        nc.vector.tensor_scalar(y_[:], y_[:], MAGIC, None, op0=Alu.subtract)
                        g_ = xp2.tile([128, XCH], F32, tag="g_")
                        nc.vector.tensor_tensor(g_[:], y_[:], t_[:], op=Alu.is_gt)
                        nc.vector.tensor_tensor(y_[:], y_[:], g_[:], op=Alu.subtract)
                        nc.vector.tensor_scalar(q_sb[:, sl], y_[:], 255.0, None, op0=Alu.min)

                # ============ phase 3: histogram ============
                junkD = pp.tile([128, FQ], BF16)
                junkA = pp.tile([128, FQ], BF16)
                with nc.named_scope("histD"):
                    for k in range(KD):
                        thr = float(k) + 0.5
                        for h2 in range(NQI):
                            ci = NQI * k + h2
                            nc.vector.tensor_scalar(
                                junkD[:], q_sb[:, h2 * FQ:(h2 + 1) * FQ], thr, None,
                                op0=Alu.is_ge, op1=Alu.add,
                                accum_out=accD[:, ci:ci + 1])
                with nc.named_scope("histA"):
                    for k in range(KD, 255):
                        j = k - KD
                        for h2 in range(NQI):
                            ci = NQI * j + h2
                            nc.scalar.activation(
                                junkA[:], q_sb[:, h2 * FQ:(h2 + 1) * FQ], Act.Sign,
                                bias=biasA[:, k:k + 1], scale=1.0,
                                accum_out=accA[:, ci:ci + 1])

            # ============ phase 4: counts + entropy ============
            cge = pp.tile([128, 255], F32)
            # sum NQI sub-accumulators per threshold (pairwise tree)
            def tree_sum(acc_tile, width):
                cur_tile, n_sub = acc_tile, NQI
                while n_sub > 1:
                    half = n_sub // 2
                    nxt = pp.tile([128, width * half], F32,
                                  tag=f"ts_{width}_{half}_{nc.next_id()}")
                    nv = nxt[:].rearrange("p (k h) -> p k h", h=half)
                    cur = cur_tile[:].rearrange("p (k h) -> p k h", h=n_sub)
                    for i2 in range(half):
                        nc.vector.tensor_tensor(nv[:, :, i2:i2 + 1],
                                                cur[:, :, 2 * i2:2 * i2 + 1],
                                                cur[:, :, 2 * i2 + 1:2 * i2 + 2], op=Alu.add)
                    cur_tile, n_sub = nxt, half
                return cur_tile
            sD = tree_sum(accD, KD)
            nc.vector.tensor_copy(cge[:, 0:KD], sD[:])
            sA = tree_sum(accA, 255 - KD)
            # each sub-acc is sum of +-1 over FQ -> count = 0.5*total_sum + NQI*FQ/2
            chalf = pp.tile([128, 1], F32)
            nc.vector.memset(chalf[:], float(NQI * FQ // 2))
            tA3v = sA[:]
            nc.vector.scalar_tensor_tensor(cge[:, KD:255], tA3v, 0.5,
                                           bcast(chalf[:], tA3v),
                                           op0=Alu.mult, op1=Alu.add)
            # combine (c, half) partition pairs -> [64, 255]
            nc.sync.dma_start(scr_cge[:], cge[:])
            cgeE = pp.tile([64, 255], F32)
            cgeO = pp.tile([64, 255], F32)
            nc.sync.dma_start(cgeE[:], dram_ap(scr_cge, 0, [[510, 64], [1, 255]]))
            nc.sync.dma_start(cgeO[:], dram_ap(scr_cge, 255, [[510, 64], [1, 255]]))
            cgeC = pp.tile([64, 255], F32)
            nc.vector.tensor_tensor(cgeC[:], cgeE[:], cgeO[:], op=Alu.add)

            nT = pp.tile([64, NBINS], F32)
            nc.vector.tensor_scalar(nT[:, 0:1], cgeC[:, 0:1], -1.0, float(N), op0=Alu.mult, op1=Alu.add)
            nc.vector.tensor_tensor(nT[:, 1:255], cgeC[:, 0:254], cgeC[:, 1:255], op=Alu.subtract)
            nc.vector.tensor_copy(nT[:, 255:256], cgeC[:, 254:255])
            nc.sync.dma_start(dbg_n[:], nT[:])

            histT = pp.tile([64, NBINS], F32)
            nc.vector.tensor_scalar(histT[:], nT[:], 1e-8, None, op0=Alu.add)
            S_ = pp.tile([64, 1], F32)
            nc.vector.tensor_reduce(S_[:], histT[:], axis=AxX, op=Alu.add)
            rS = pp.tile([64, 1], F32)
            nc.vector.reciprocal(rS[:], S_[:])
            probT = pp.tile([64, NBINS], F32)
            nc.vector.tensor_scalar(probT[:], histT[:], rS[:], None, op0=Alu.mult)
            pe_ = pp.tile([64, NBINS], F32)
            nc.vector.tensor_scalar(pe_[:], probT[:], 1e-8, None, op0=Alu.add)
            lnT = pp.tile([64, NBINS], F32)
            nc.scalar.activation(lnT[:], pe_[:], Act.Ln, bias=zb[0:64, :], scale=1.0)
            termT = pp.tile([64, NBINS], F32)
            nc.vector.tensor_tensor(termT[:], probT[:], lnT[:], op=Alu.mult)
            actT = pp.tile([64, 1], F32)
            nc.vector.tensor_reduce(actT[:], termT[:], axis=AxX, op=Alu.add, negate=True)
            nc.sync.dma_start(dbg_act[:], actT[:])

            # ============ phase 5: top-16 ============
            nc.sync.dma_start(dram_ap(scr_act, 0, [[1, 64]]), actT[:])
            act64 = pp.tile([1, 64], F32)
            nc.sync.dma_start(act64[:], dram_ap(scr_act, 0, [[64, 1], [1, 64]]))
            idx16 = pp.tile([1, 16], U32)
            m8 = pp.tile([1, 8], F32)
            nc.vector.max(m8[:], act64[:])
            nc.vector.max_index(idx16[:, 0:8], m8[:], act64[:])
            act64b = pp.tile([1, 64], F32)
            nc.vector.match_replace(act64b[:], m8[:], act64[:], -3.0e38)
            m8b = pp.tile([1, 8], F32)
            nc.vector.max(m8b[:], act64b[:])
            nc.vector.max_index(idx16[:, 8:16], m8b[:], act64b[:])
            act64c = pp.tile([1, 64], F32)
            nc.vector.match_replace(act64c[:], m8b[:], act64b[:], -3.0e38)
            nc.sync.dma_start(dbg_idx[:], idx16[:])

            # ============ phase 6: selection metadata ============
            idx16f = pp.tile([1, 16], F32)
            nc.vector.tensor_copy(idx16f[:], idx16[:])
            nc.sync.dma_start(dram_ap(scr_idx, 0, [[1, 16]]), idx16f[:])
            idx16T = pp.tile([16, 1], F32)
            nc.sync.dma_start(idx16T[:], dram_ap(scr_idx, 0, [[1, 16], [16, 1]]))
            goffF8 = pp.tile([16, 16], F32)
            for j in range(16):
                nc.vector.tensor_scalar(goffF8[:, j:j + 1], idx16T[:], 16.0, float(j),
                                        op0=Alu.mult, op1=Alu.add)
            goffI8 = pp.tile([16, 16], I32)
            nc.vector.tensor_copy(goffI8[:], goffF8[:])

            mask01 = pp.tile([1, 64], F32)
            nc.vector.tensor_scalar(mask01[:], act64c[:], -1.0e38, None, op0=Alu.is_le)
            rank = pp.tile([1, 64], F32)
            nc.vector.tensor_tensor_scan(rank[:], mask01[:], z64[:], 0.0, op0=Alu.add, op1=Alu.add)
            sl1 = pp.tile([1, 64], F32)
            nc.vector.tensor_tensor(sl1[:], colio[:], rank[:], op=Alu.subtract)
            sl2 = pp.tile([1, 64], F32)
            nc.vector.tensor_scalar(sl2[:], sl1[:], 64.0, None, op0=Alu.add)
            slotf = pp.tile([1, 64], F32)
            nc.vector.scalar_tensor_tensor(slotf[:], mask01[:], 100000.0, sl2[:],
                                           op0=Alu.mult, op1=Alu.add)
            slot8 = pp.tile([1, 64], F32)
            nc.vector.tensor_scalar(slot8[:], slotf[:], 16.0, None, op0=Alu.mult)
            nc.sync.dma_start(dram_ap(scr_slot, 0, [[1, 64]]), slot8[:])
            slot8T = pp.tile([64, 1], F32)
            nc.sync.dma_start(slot8T[:], dram_ap(scr_slot, 0, [[1, 64], [64, 1]]))
            offsF = pp.tile([64, 16], F32)
            for j in range(16):
                nc.vector.tensor_scalar(offsF[:, j:j + 1], slot8T[:], float(j), None, op0=Alu.add)
            offsI = pp.tile([64, 16], I32)
            nc.vector.tensor_copy(offsI[:], offsF[:])

            # ============ phase 7: selected gather + conv ============
            with tc.tile_pool(name="convp", bufs=1) as cp:
                selb = cp.tile([16, N], BF16)
                GCH = N // 16
                with tc.tile_pool(name="gathp", bufs=1) as gp:
                    for j in range(16):
                        ga = gp.tile([16, GCH], F32, tag="ga")
                        nc.gpsimd.indirect_dma_start(
                            out=ga[:], out_offset=None,
                            in_=x_ext[:].rearrange("c (t m) -> (c t) m", t=16),
                            in_offset=bass.IndirectOffsetOnAxis(ap=goffI8[:, j:j + 1], axis=0))
                        nc.vector.tensor_copy(selb[:, j * GCH:(j + 1) * GCH], ga[:])

                with tc.tile_pool(name="psum", bufs=4, space="PSUM") as psp, \
                     tc.tile_pool(name="stage", bufs=2) as stp:
                    RPS = 8
                    for blk in range(32):
                        stage = stp.tile([OC, RPS * W], F32, tag="stage")
                        for yy in range(RPS):
                            y = blk * RPS + yy
                            ps = psp.tile([OC, W], F32, tag="ps")
                            taps = []
                            for dy in (-1, 0, 1):
                                ys = y + dy
                                if 0 <= ys < H:
                                    for dx in (-1, 0, 1):
                                        taps.append((dy, dx, ys))
                            for ti, (dy, dx, ys) in enumerate(taps):
                                t_idx = (dy + 1) * 3 + (dx + 1)
                                if dx == -1:
                                    rhs = selb[:, ys * W:ys * W + (W - 1)]
                                    outp = ps[:, 1:W]
                                elif dx == 1:
                                    rhs = selb[:, ys * W + 1:ys * W + W]
                                    outp = ps[:, 0:W - 1]
                                else:
                                    rhs = selb[:, ys * W:ys * W + W]
                                    outp = ps[:, 0:W]
                                nc.tensor.matmul(outp, wsb[:, t_idx * OC:(t_idx + 1) * OC], rhs,
                                                 start=(ti == 0), stop=(ti == len(taps) - 1))
                            if yy % 2 == 0:
                                nc.vector.tensor_scalar(stage[:, yy * W:(yy + 1) * W], ps[:],
                                                        biasC[:], None, op0=Alu.add)
                            else:
                                nc.scalar.activation(stage[:, yy * W:(yy + 1) * W], ps[:],
                                                     Act.Identity, bias=biasC[:], scale=1.0)
                        nc.sync.dma_start(
                            out_ext[0:OC, blk * RPS * W:(blk + 1) * RPS * W], stage[:])

            # ============ phase 8: unselected passthrough ============
            with tc.tile_pool(name="upass", bufs=2) as up:
                UCH = N // 16
                for j in range(16):
                    ub = up.tile([64, UCH], F32, tag="ub")
                    nc.sync.dma_start(ub[:], x_ext[:, j * UCH:(j + 1) * UCH])
                    nc.gpsimd.indirect_dma_start(
                        out=out_ext[:].rearrange("c (b m) -> (c b) m", b=16),
                        out_offset=bass.IndirectOffsetOnAxis(ap=offsI[:, j:j + 1], axis=0),
                        in_=ub[:], in_offset=None,
                        bounds_check=C_OUT * 16 - 1, oob_is_err=False)
    nc.compile()
    return nc


_CACHED = {}


def _get_nc():
    if "nc" not in _CACHED:
        _CACHED["nc"] = build()
    return _CACHED["nc"]


def make_inputs_per_core(x, weight, bias):
    x = np.ascontiguousarray(x, dtype=np.float32)
    weight = np.asarray(weight, dtype=np.float32)
    bias = np.asarray(bias, dtype=np.float32)
    wt = np.ascontiguousarray(np.transpose(weight, (1, 2, 3, 0)).reshape(16, 9 * OC))
    biasT = np.ascontiguousarray(bias.reshape(OC, 1))
    biasA = np.ascontiguousarray(
        np.broadcast_to(-(np.arange(255, dtype=np.float32) + 0.5), (128, 255)))
    blkvec = np.ascontiguousarray((np.arange(128, dtype=np.float32) // 16).reshape(128, 1))
    colio = np.ascontiguousarray(np.arange(64, dtype=np.float32).reshape(1, 64))
    maps = []
    for b in range(B):
        maps.append({
            "x": np.ascontiguousarray(x[b].reshape(C, N)),
            "w": wt, "bias": biasT, "biasA": biasA,
            "blkvec": blkvec, "colio": colio,
        })
    return maps


LAST_RESULT = {}


def kernel(x, weight, bias):
    nc = _get_nc()
    maps = make_inputs_per_core(x, weight, bias)
    trace = bool(int(os.environ.get("KERNEL_TRACE", "0")))
    if trace:
        sys.path.insert(0, os.path.dirname(os.path.abspath(__file__)))
        try:
            import profhook
            profhook.install()
        except Exception:
            trace = False
    res = bu.run_bass_kernel_spmd(nc, maps, list(range(8)), trace=trace)
    LAST_RESULT["res"] = res
    out = np.stack([res.results[i]["out"].reshape(C_OUT, H, W) for i in range(B)])
    return out


if __name__ == "__main__":
    import reference as R
    inputs = R.setup_inputs()
    out = kernel(np.asarray(inputs["x"]), np.asarray(inputs["weight"]),
                 np.asarray(inputs["bias"]))
    print("out shape:", out.shape)
